# revision 2
# baseline (speedup 1.0000x reference)
"""Trainium2 Bass kernel v2 for 2-layer GCN (GCNConv + ELU, x2), 8 NeuronCores.

Improvements over v1 baseline:
  - Balanced schedule: per-core greedy node->tile assignment targets <=512
    edges per (tile, src-bucket) segment for 90 "small" tiles + 8 "big"
    overflow tiles (7 chunks), shrinking chunk padding ~25% -> ~5%.
  - S one-hot built at DVE 2x rate: iota materialized with innermost chunk
    dim (io_mat[P, P, NCHMAX]), S stored [P, P, nch] (innermost k packed on
    both operands), matmul takes strided rhs S[:, :, k].
  - Transposed feature-table layout gT[p, t, f]: prep does one big read +
    one big write; conv preloads own-core gT once (diag matmul lhsT reads
    SBUF directly, no per-tile DMA); epilogue stages per-group outputs and
    writes [128, GT*feat] per group. Gather row id (c,t,p) -> c*12544 +
    p*98 + t keeps 25088-row buckets aligned to core pairs.
"""
import dataclasses
import numpy as np
import concourse.bacc as bacc
import concourse.mybir as mybir
import concourse.tile as tile

P = 128
N_CORES = 8
N_NODES = 100000
TPC = 98             # dst tiles per core
GT = 6               # dst tiles per PSUM group
BROWS = 25088        # src bucket rows (int16 idx range; = 2 cores' rows)
N_BUCKETS = 4
BIG_TILES = 8        # overflow tiles per core, one per early group
BIG_TILE_POS = [6 * g + 5 for g in range(BIG_TILES)]  # local tile indices
SMALL_CAP = 512      # 4 chunks
BIG_CAP = 896        # 7 chunks
IN_DIM, HID_DIM, OUT_DIM = 128, 128, 64


def _balance_tiles(vecs, nvalid):
    """Greedy node->tile assignment, vectorized across cores.

    vecs: [N_CORES, npc, 4] per-node in-degree-by-bucket, rows sorted by
    descending total in-degree; nvalid: valid rows per core.
    Returns tile_local [N_CORES, npc].

    The BIG_TILES tiles at BIG_TILE_POS absorb the heaviest 8*P nodes
    (lowering the small-tile per-bucket mean to ~490 < SMALL_CAP=512 with
    margin), then a makespan greedy packs the rest into small tiles so
    every (tile, bucket) count stays below 512 = 4 chunks.
    """
    npc = vecs.shape[1]
    loads = np.zeros((N_CORES, TPC, 4), dtype=np.int64)
    counts = np.zeros((N_CORES, TPC), dtype=np.int64)
    out = np.zeros((N_CORES, npc), dtype=np.int64)
    cid = np.arange(N_CORES)
    big = np.array(BIG_TILE_POS)
    small = np.array([t for t in range(TPC) if t not in set(BIG_TILE_POS)])
    nbig = len(big) * P
    for r in range(npc):
        v = vecs[:, r, :]                          # [8, 4]
        tiles = big if r < nbig else small
        after = loads[:, tiles, :] + v[:, None, :]  # [8, ntiles, 4]
        # makespan greedy: minimize the resulting max per-bucket count,
        # tie-broken toward emptier tiles for the 128-node cap
        pen = after.max(axis=2) * 64 + counts[:, tiles]
        pen = pen + np.where(counts[:, tiles] >= P, 1 << 40, 0)
        t = tiles[np.argmin(pen, axis=1)]
        out[:, r] = t
        loads[cid, t] += v
        counts[cid, t] += 1
    return out


def build_schedule(edge_index, n_nodes):
    src0 = np.asarray(edge_index[0], dtype=np.int64).astype(np.int32)
    dst0 = np.asarray(edge_index[1], dtype=np.int64).astype(np.int32)
    deg = (np.bincount(dst0, minlength=n_nodes) + 1).astype(np.float64)
    dinv = np.where(deg > 0, 1.0 / np.sqrt(deg), 0.0).astype(np.float32)

    n_slots = TPC * P * N_CORES
    assert n_slots >= n_nodes

    # ---- phase 1: provisional core assignment (snake-deal by degree) so
    # every core gets ~equal total edges; a node's CORE fixes its src bucket
    # (bucket = core//2), so phase 2 only shuffles nodes within a core.
    # Capacity 12544/core > ceil(100000/8), so the snake never overflows.
    order_by_deg = np.argsort(-deg, kind="stable")
    snake = np.r_[np.arange(N_CORES), np.arange(N_CORES)[::-1]]
    pattern = np.tile(snake, n_nodes // (2 * N_CORES) + 1)[:n_nodes]
    core_of_node = np.empty(n_nodes, dtype=np.int64)
    core_of_node[order_by_deg] = pattern
    # ---- phase 2: per-core balanced tile assignment by in-degree-per-bucket
    bucket_of_node = core_of_node // 2  # src bucket of a node (by table layout)
    ib = np.zeros((n_nodes, 4), dtype=np.int64)
    np.add.at(ib, (dst0, bucket_of_node[src0]), 1)
    # self-loops handled as diag (not edges) - not counted here
    nodes_by_core = [order_by_deg[core_of_node[order_by_deg] == c]
                     for c in range(N_CORES)]
    npcs = [len(n) for n in nodes_by_core]
    assert max(npcs) <= TPC * P
    vecs = np.zeros((N_CORES, max(npcs), 4), dtype=np.int64)
    nodemat = np.zeros((N_CORES, max(npcs)), dtype=np.int64)
    for c in range(N_CORES):
        nodemat[c, :npcs[c]] = nodes_by_core[c]
        vecs[c, :npcs[c]] = ib[nodes_by_core[c]]
    tile_local = _balance_tiles(vecs, nodemat)

    # ---- slots: node -> (core, tile_local, p)
    slot_of = np.empty(n_nodes, dtype=np.int64)
    tfill = np.zeros((N_CORES, TPC), dtype=np.int64)
    for c in range(N_CORES):
        tl = tile_local[c, :npcs[c]]
        nodes = nodemat[c, :npcs[c]]
        for i in range(npcs[c]):
            t = tl[i]
            slot_of[nodes[i]] = (c * TPC + t) * P + tfill[c, t]
            tfill[c, t] += 1
    assert tfill.max() <= P

    src = slot_of[src0].astype(np.int64)
    dst = slot_of[dst0].astype(np.int64)
    dinv_slot = np.zeros(n_slots, dtype=np.float32)
    dinv_slot[slot_of] = dinv

    gtile = dst >> 7
    core = gtile // TPC
    tl = gtile % TPC
    dstp = dst & (P - 1)
    # table row of a node slot (c, t, p): c*12544 + p*TPC + t
    sc = src // (TPC * P)
    st = (src // P) % TPC
    sp = src & (P - 1)
    table_row = sc * (TPC * P) + sp * TPC + st
    bucket = table_row // BROWS  # == sc // 2
    idx_rel_all = (table_row - bucket * BROWS).astype(np.int32)

    # ---- counts and uniform chunk schedule
    counts = np.zeros((N_CORES, TPC, N_BUCKETS), dtype=np.int64)
    np.add.at(counts, (core, tl, bucket), 1)
    nchunk_u = (np.ceil(counts.max(axis=0) / P)).astype(np.int64)  # [TPC, 4]
    nchunk_u = np.maximum(nchunk_u, 1)

    n_groups = (TPC + GT - 1) // GT
    chunk_tile = []
    gb_meta = []   # (g, b, chunk_start, nch, idx_col_start)
    slot_base = {}
    cpos = 0
    col_off = 0
    gb_nchunks = np.zeros((n_groups, N_BUCKETS), dtype=np.int64)
    for g in range(n_groups):
        t0, t1 = g * GT, min((g + 1) * GT, TPC)
        for b in range(N_BUCKETS):
            nch = 0
            for t in range(t0, t1):
                slot_base[(t, b)] = (cpos + nch) * P
                for _ in range(int(nchunk_u[t, b])):
                    chunk_tile.append(t)
                nch += int(nchunk_u[t, b])
            gb_meta.append((g, b, cpos, nch, col_off))
            gb_nchunks[g, b] = nch
            col_off += nch * P // 16
            cpos += nch
    nct = cpos
    chunk_tile = np.array(chunk_tile, dtype=np.int32)

    # ---- per-core edge placement
    order = np.lexsort((bucket, tl, core))
    s_c, s_t, s_b = core[order], tl[order], bucket[order]
    s_idx, s_dstp = idx_rel_all[order], dstp[order]

    idx_rel = np.zeros((N_CORES, nct * P), dtype=np.int16)  # pad: row 0, S=0
    dstloc = np.full((N_CORES, nct * P), 300.0, dtype=np.float16)
    key = (s_c * TPC + s_t) * N_BUCKETS + s_b
    startd = np.r_[True, key[1:] != key[:-1]]
    run_id = np.cumsum(startd) - 1
    run_start = np.nonzero(startd)[0]
    within = np.arange(len(key)) - run_start[run_id]
    base = np.array([slot_base[(int(t), int(b))]
                     for t, b in zip(s_t[startd], s_b[startd])])
    gpos = base[run_id] + within
    idx_rel[s_c, gpos] = s_idx.astype(np.int16)
    dstloc[s_c, gpos] = s_dstp.astype(np.float16)

    # wrapped idx layout: per (g,b) call, idx i -> [16 partitions, i//16], x8
    idx_cols_total = nct * P // 16
    idx_wrapped = np.zeros((N_CORES, P, idx_cols_total), dtype=np.int16)
    for (g, b, cpos0, nch, col0) in gb_meta:
        if nch == 0:
            continue
        ni = nch * P
        span = slice(cpos0 * P, cpos0 * P + ni)
        blk = idx_rel[:, span].reshape(N_CORES, ni // 16, 16)
        w = np.transpose(blk, (0, 2, 1))
        idx_wrapped[:, :, col0:col0 + ni // 16] = np.tile(w, (1, 8, 1))

    dstloc_T = np.transpose(dstloc.reshape(N_CORES, nct, P), (0, 2, 1)).copy()
    dinv_T = dinv_slot.reshape(N_CORES, TPC, P).transpose(0, 2, 1).copy()

    nchmax = int(gb_nchunks.max())
    io_mat = np.tile(np.arange(P, dtype=np.float16)[None, :, None],
                     (P, 1, nchmax)).copy()

    pad = nct * P / (len(src) / N_CORES) - 1
    return dict(
        dinv=dinv_slot, dinv_T=dinv_T, io_mat=io_mat, slot_of=slot_of,
        idx_wrapped=idx_wrapped, dstloc_T=dstloc_T,
        chunk_tile=chunk_tile, gb_meta=gb_meta, gb_nchunks=gb_nchunks,
        n_groups=n_groups, n_chunks_total=nct, nchmax=nchmax, pad=pad,
    )


def build_prep_kernel(feat, R=1, nsplit=7):
    """gT[p, t, f] = fp16(dinv[t,p] * xT[p, t, f]); pipelined big DMAs."""
    nc = bacc.Bacc("TRN2")
    nt = TPC
    q = nt // nsplit
    assert nt % nsplit == 0
    x = nc.dram_tensor("x", [P, nt * feat], mybir.dt.float16, kind="ExternalInput")
    dinvT = nc.dram_tensor("dinvT", [P, nt], mybir.dt.float32, kind="ExternalInput")
    g = nc.dram_tensor("g", [P, nt * feat], mybir.dt.float16, kind="ExternalOutput")
    with tile.TileContext(nc) as tc:
        with tc.tile_pool(name="sb", bufs=3) as pool, \
             tc.tile_pool(name="cst", bufs=1) as cpool:
            dv = cpool.tile([P, nt], mybir.dt.float32)
            nc.sync.dma_start(dv[:], dinvT[:])
            for _ in range(R):
                for s in range(nsplit):
                    t0 = s * q
                    xt = pool.tile([P, q, feat], mybir.dt.float16, tag="x")
                    nc.sync.dma_start(xt[:], x[:, t0*feat:(t0+q)*feat])
                    gt = pool.tile([P, q, feat], mybir.dt.float16, tag="g")
                    for t in range(q):
                        nc.vector.tensor_scalar(gt[:, t, :], xt[:, t, :],
                                                dv[:, t0+t:t0+t+1], None,
                                                mybir.AluOpType.mult)
                    nc.sync.dma_start(g[:, t0*feat:(t0+q)*feat], gt[:])
    nc.finalize()
    return nc


def build_conv_kernel(sched, feat_in, feat_out, out_fp16_scaled, R=1,
                      msg_bufs=8, s_bufs=4, ep_bufs=6, st_bufs=3):
    """One GCN conv layer (aggregate-first), v2.

    inputs: g [n_rows, feat_in] fp16 (table layout, row (c,t,p) at
            c*12544 + p*98 + t), gownT [128, TPC*feat_in] fp16,
            W [feat_in, feat_out] fp16, dinvT [128, TPC] fp32,
            io_mat [128, 128*nchmax] fp16, idxs [128, idx_cols] int16,
            dstlocT [128, nct] fp16
    output: outT [128, TPC*feat_out] (fp16 scaled by dinv, or fp32)
    """
    n_groups = sched["n_groups"]
    nct = sched["n_chunks_total"]
    chunk_tile = sched["chunk_tile"]
    gb_meta = sched["gb_meta"]
    nchmax = sched["nchmax"]
    idx_cols = sched["idx_wrapped"].shape[2]
    n_rows = TPC * P * N_CORES

    first_chunk = {}
    last_chunk = {}
    for ci, t in enumerate(chunk_tile):
        t = int(t)
        if t not in first_chunk:
            first_chunk[t] = ci
        last_chunk[t] = ci

    out_dtype = mybir.dt.float16 if out_fp16_scaled else mybir.dt.float32

    nc = bacc.Bacc("TRN2", num_swdge_queues=4)
    g = nc.dram_tensor("g", [n_rows, feat_in], mybir.dt.float16, kind="ExternalInput")
    gownT_t = nc.dram_tensor("gownT", [P, TPC * feat_in], mybir.dt.float16,
                             kind="ExternalInput")
    W = nc.dram_tensor("W", [feat_in, feat_out], mybir.dt.float16, kind="ExternalInput")
    dinvT = nc.dram_tensor("dinvT", [P, TPC], mybir.dt.float32, kind="ExternalInput")
    iomat_t = nc.dram_tensor("iomat", [P, P * nchmax], mybir.dt.float16,
                             kind="ExternalInput")
    ident_t = nc.dram_tensor("ident", [P, P], mybir.dt.float16,
                             kind="ExternalInput")
    idxs = nc.dram_tensor("idxs", [P, idx_cols], mybir.dt.int16, kind="ExternalInput")
    dstlocT = nc.dram_tensor("dstlocT", [P, nct], mybir.dt.float16,
                             kind="ExternalInput")
    out = nc.dram_tensor("out", [P, TPC * feat_out], out_dtype, kind="ExternalOutput")

    with tile.TileContext(nc) as tc:
        with tc.tile_pool(name="cst", bufs=1) as cpool, \
             tc.tile_pool(name="msg", bufs=msg_bufs) as mpool, \
             tc.tile_pool(name="sS", bufs=s_bufs) as spool, \
             tc.tile_pool(name="agg", bufs=1, space="PSUM") as apool, \
             tc.tile_pool(name="ops", bufs=2, space="PSUM") as opool, \
             tc.tile_pool(name="eps", bufs=ep_bufs) as epool, \
             tc.tile_pool(name="st", bufs=st_bufs) as stpool:
            w_sb = cpool.tile([feat_in, feat_out], mybir.dt.float16)
            nc.sync.dma_start(w_sb[:], W[:])
            dv = cpool.tile([P, TPC], mybir.dt.float32)
            nc.sync.dma_start(dv[:], dinvT[:])
            iom = cpool.tile([P, P, nchmax], mybir.dt.float16)
            nc.sync.dma_start(iom[:], iomat_t[:])
            idn = cpool.tile([P, P], mybir.dt.float16)
            nc.sync.dma_start(idn[:], ident_t[:])
            ix = cpool.tile([P, idx_cols], mybir.dt.int16)
            nc.sync.dma_start(ix[:], idxs[:])
            dl = cpool.tile([P, nct], mybir.dt.float16)
            nc.sync.dma_start(dl[:], dstlocT[:])
            gown = cpool.tile([P, TPC, feat_in], mybir.dt.float16)
            nc.sync.dma_start(gown[:], gownT_t[:])

            for _ in range(R):
                for gi in range(n_groups):
                    t0 = gi * GT
                    t1 = min((gi + 1) * GT, TPC)
                    banks = [apool.tile([P, 512], mybir.dt.float32, tag=f"agg{k}",
                                        name=f"aggb_{gi}_{k}")
                             for k in range(t1 - t0)]

                    def agg_slice(t):
                        return banks[t - t0][:, :P]

                    # self-loop diagonal: aggT[:, p] = gown[p, t, :]^T via PE
                    # transpose (lhsT read straight from preloaded SBUF)
                    for t in range(t0, t1):
                        nc.tensor.matmul(agg_slice(t), lhsT=gown[:, t, :],
                                         rhs=idn[:], start=True, stop=False)

                    for b in range(N_BUCKETS):
                        _, _, c_start, nch, col0 = gb_meta[gi * N_BUCKETS + b]
                        if nch == 0:
                            continue
                        msg = mpool.tile([P, nch, feat_in], mybir.dt.float16,
                                         tag="msg")
                        base = b * BROWS
                        rows = min(BROWS, n_rows - base)
                        nc.gpsimd.dma_gather(
                            msg[:], g[base:base + rows, :],
                            ix[:, col0:col0 + nch * P // 16],
                            nch * P, nch * P, feat_in,
                            single_packet=False,
                            queue_num=(gi * N_BUCKETS + b) % 4,
                        )
                        # S2[e, d, k] = (d == dstloc[e, c_start+k]); both
                        # operands innermost-packed -> DVE 2x
                        S2 = spool.tile([P, P, nch], mybir.dt.float16, tag="S")
                        dl_ap = dl[:, c_start:c_start + nch]
                        dl_r = dataclasses.replace(
                            dl_ap, ap=[dl_ap.ap[0], [0, P], dl_ap.ap[1]])
                        nc.vector.tensor_tensor(
                            S2[:], iom[:, :, :nch], dl_r,
                            mybir.AluOpType.is_equal)
                        for k in range(nch):
                            ci = c_start + k
                            t = int(chunk_tile[ci])
                            nc.tensor.matmul(
                                agg_slice(t), lhsT=msg[:, k, :],
                                rhs=S2[:, :, k],
                                start=False, stop=(ci == last_chunk[t]))

                    # epilogue: stage group outputs, one DMA per group
                    ot = stpool.tile([P, t1 - t0, feat_out], out_dtype, tag="ot")
                    for t in range(t0, t1):
                        aggsb = epool.tile([P, P], mybir.dt.float16, tag="aggsb")
                        nc.vector.tensor_copy(aggsb[:], agg_slice(t))
                        ops = opool.tile([P, feat_out], mybir.dt.float32, tag="ops")
                        nc.tensor.matmul(ops[:], lhsT=aggsb[:], rhs=w_sb[:],
                                         start=True, stop=True)
                        dvt = dv[:, t:t+1]
                        e = epool.tile([P, feat_out], mybir.dt.float32, tag="e")
                        nc.scalar.activation(e[:], ops[:],
                                             mybir.ActivationFunctionType.Exp,
                                             scale=dvt)
                        r = epool.tile([P, feat_out], mybir.dt.float32, tag="r")
                        nc.scalar.activation(r[:], e[:],
                                             mybir.ActivationFunctionType.Relu,
                                             bias=1.0, scale=-1.0)
                        p = epool.tile([P, feat_out], mybir.dt.float32, tag="p")
                        nc.scalar.activation(p[:], ops[:],
                                             mybir.ActivationFunctionType.Relu,
                                             scale=dvt)
                        if out_fp16_scaled:
                            elu = epool.tile([P, feat_out], mybir.dt.float32,
                                             tag="elu")
                            nc.vector.tensor_tensor(elu[:], p[:], r[:],
                                                    mybir.AluOpType.subtract)
                            nc.vector.tensor_scalar(ot[:, t - t0, :], elu[:],
                                                    dvt, None,
                                                    mybir.AluOpType.mult)
                        else:
                            nc.vector.tensor_tensor(ot[:, t - t0, :], p[:], r[:],
                                                    mybir.AluOpType.subtract)
                    nc.sync.dma_start(
                        out[:, t0 * feat_out:t1 * feat_out], ot[:])
    nc.finalize()
    return nc


import sys as _sys
import types as _types


def _ensure_axon_stub():
    try:
        import antenv.axon_hooks  # noqa
    except ModuleNotFoundError:
        try:
            import antenv
        except ModuleNotFoundError:
            antenv = _types.ModuleType("antenv")
            _sys.modules["antenv"] = antenv
        import antenv
        m = _types.ModuleType("antenv.axon_hooks")
        m.get_axon_ntff_profile_hook = lambda: None
        _sys.modules["antenv.axon_hooks"] = m
        antenv.axon_hooks = m


def _to_xT(x_rows):
    """[12544, feat] row-major (t,p) -> [128, TPC*feat] partition-major."""
    feat = x_rows.shape[1]
    return np.ascontiguousarray(
        x_rows.reshape(TPC, P, feat).transpose(1, 0, 2).reshape(P, TPC * feat))


def _from_outT(outT, feat):
    """[128, TPC*feat] -> [12544, feat] row-major (t,p)."""
    return np.ascontiguousarray(
        outT.reshape(P, TPC, feat).transpose(1, 0, 2).reshape(TPC * P, feat))


def kernel(x, edge_index, W1, b1, W2, b2):
    _ensure_axon_stub()
    from concourse.bass_utils import run_bass_kernel_spmd

    x = np.asarray(x, dtype=np.float32)
    edge_index = np.asarray(edge_index)
    W1 = np.asarray(W1, dtype=np.float32)
    W2 = np.asarray(W2, dtype=np.float32)
    assert np.all(np.asarray(b1) == 0) and np.all(np.asarray(b2) == 0)

    sched = build_schedule(edge_index, N_NODES)
    slot_of = sched["slot_of"]
    rows_pc = TPC * P
    cores = list(range(N_CORES))

    # ---- launch A: gT = fp16(dinv * x), node-sharded, transposed layout
    x_pad = np.zeros((rows_pc * N_CORES, IN_DIM), np.float32)
    x_pad[slot_of] = x
    nc_a = build_prep_kernel(IN_DIM)
    in_a = [{"x": _to_xT(x_pad[c*rows_pc:(c+1)*rows_pc]).astype(np.float16),
             "dinvT": sched["dinv_T"][c]} for c in cores]
    res_a = run_bass_kernel_spmd(nc_a, in_a, core_ids=cores, trace=False)
    gT = [res_a.results[c]["g"] for c in cores]  # [128, TPC*IN_DIM] each
    # global gather table: core c rows at c*12544, row (c,t,p) = c*12544+p*98+t
    g1 = np.concatenate([gt.reshape(rows_pc, IN_DIM) for gt in gT])

    ident = np.eye(P, dtype=np.float16)
    common = lambda c: {"dinvT": sched["dinv_T"][c],
                        "iomat": sched["io_mat"].reshape(P, -1),
                        "ident": ident,
                        "idxs": sched["idx_wrapped"][c],
                        "dstlocT": sched["dstloc_T"][c]}

    # ---- launch B: conv1 -> fp16(dinv * elu(.)) in gT layout
    nc_b = build_conv_kernel(sched, IN_DIM, HID_DIM, out_fp16_scaled=True)
    in_b = [{"g": g1, "gownT": gT[c], "W": W1.astype(np.float16), **common(c)}
            for c in cores]
    res_b = run_bass_kernel_spmd(nc_b, in_b, core_ids=cores, trace=False)
    g2T = [res_b.results[c]["out"] for c in cores]
    g2 = np.concatenate([gt.reshape(rows_pc, HID_DIM) for gt in g2T])

    # ---- launch C: conv2 -> fp32 elu(.)
    nc_c = build_conv_kernel(sched, HID_DIM, OUT_DIM, out_fp16_scaled=False)
    in_c = [{"g": g2, "gownT": g2T[c], "W": W2.astype(np.float16), **common(c)}
            for c in cores]
    res_c = run_bass_kernel_spmd(nc_c, in_c, core_ids=cores, trace=False)
    out = np.concatenate([_from_outT(res_c.results[c]["out"], OUT_DIM)
                          for c in cores])
    # rows are in (c,t,p) slot order = slot id; un-permute
    return np.ascontiguousarray(out[slot_of].astype(np.float32))


# revision 4
# speedup vs baseline: 3.0788x; 3.0788x over previous
"""Trainium2 Bass kernel v2 for 2-layer GCN (GCNConv + ELU, x2), 8 NeuronCores.

Improvements over v1 baseline:
  - Balanced schedule: per-core greedy node->tile assignment targets <=512
    edges per (tile, src-bucket) segment for 90 "small" tiles + 8 "big"
    overflow tiles (7 chunks), shrinking chunk padding ~25% -> ~5%.
  - S one-hot built at DVE 2x rate: iota materialized with innermost chunk
    dim (io_mat[P, P, NCHMAX]), S stored [P, P, nch] (innermost k packed on
    both operands), matmul takes strided rhs S[:, :, k].
  - Transposed feature-table layout gT[p, t, f]: prep does one big read +
    one big write; conv preloads own-core gT once (diag matmul lhsT reads
    SBUF directly, no per-tile DMA); epilogue stages per-group outputs and
    writes [128, GT*feat] per group. Gather row id (c,t,p) -> c*12544 +
    p*98 + t keeps 25088-row buckets aligned to core pairs.
"""
import dataclasses
import numpy as np
import concourse.bacc as bacc
import concourse.mybir as mybir
import concourse.tile as tile

P = 128
N_CORES = 8
N_NODES = 100000
TPC = 98             # dst tiles per core
GT = 6               # dst tiles per PSUM group
BROWS = 25088        # src bucket rows (int16 idx range; = 2 cores' rows)
N_BUCKETS = 4
BIG_TILES = 8        # overflow tiles per core, one per early group
BIG_TILE_POS = [6 * g + 5 for g in range(BIG_TILES)]  # local tile indices
SMALL_CAP = 512      # 4 chunks
BIG_CAP = 896        # 7 chunks
IN_DIM, HID_DIM, OUT_DIM = 128, 128, 64


def _balance_tiles(vecs, nvalid):
    """Greedy node->tile assignment, vectorized across cores.

    vecs: [N_CORES, npc, 4] per-node in-degree-by-bucket, rows sorted by
    descending total in-degree; nvalid: valid rows per core.
    Returns tile_local [N_CORES, npc].

    The BIG_TILES tiles at BIG_TILE_POS absorb the heaviest 8*P nodes
    (lowering the small-tile per-bucket mean to ~490 < SMALL_CAP=512 with
    margin), then a makespan greedy packs the rest into small tiles so
    every (tile, bucket) count stays below 512 = 4 chunks.
    """
    npc = vecs.shape[1]
    loads = np.zeros((N_CORES, TPC, 4), dtype=np.int64)
    counts = np.zeros((N_CORES, TPC), dtype=np.int64)
    out = np.zeros((N_CORES, npc), dtype=np.int64)
    cid = np.arange(N_CORES)
    big = np.array(BIG_TILE_POS)
    small = np.array([t for t in range(TPC) if t not in set(BIG_TILE_POS)])
    nbig = len(big) * P
    for r in range(npc):
        v = vecs[:, r, :]                          # [8, 4]
        tiles = big if r < nbig else small
        after = loads[:, tiles, :] + v[:, None, :]  # [8, ntiles, 4]
        # makespan greedy: minimize the resulting max per-bucket count,
        # tie-broken toward emptier tiles for the 128-node cap
        pen = after.max(axis=2) * 64 + counts[:, tiles]
        pen = pen + np.where(counts[:, tiles] >= P, 1 << 40, 0)
        t = tiles[np.argmin(pen, axis=1)]
        out[:, r] = t
        loads[cid, t] += v
        counts[cid, t] += 1
    return out


def build_schedule(edge_index, n_nodes):
    src0 = np.asarray(edge_index[0], dtype=np.int64).astype(np.int32)
    dst0 = np.asarray(edge_index[1], dtype=np.int64).astype(np.int32)
    deg = (np.bincount(dst0, minlength=n_nodes) + 1).astype(np.float64)
    dinv = np.where(deg > 0, 1.0 / np.sqrt(deg), 0.0).astype(np.float32)

    n_slots = TPC * P * N_CORES
    assert n_slots >= n_nodes

    # ---- phase 1: provisional core assignment (snake-deal by degree) so
    # every core gets ~equal total edges; a node's CORE fixes its src bucket
    # (bucket = core//2), so phase 2 only shuffles nodes within a core.
    # Capacity 12544/core > ceil(100000/8), so the snake never overflows.
    order_by_deg = np.argsort(-deg, kind="stable")
    snake = np.r_[np.arange(N_CORES), np.arange(N_CORES)[::-1]]
    pattern = np.tile(snake, n_nodes // (2 * N_CORES) + 1)[:n_nodes]
    core_of_node = np.empty(n_nodes, dtype=np.int64)
    core_of_node[order_by_deg] = pattern
    # ---- phase 2: per-core balanced tile assignment by in-degree-per-bucket
    bucket_of_node = core_of_node // 2  # src bucket of a node (by table layout)
    ib = np.zeros((n_nodes, 4), dtype=np.int64)
    np.add.at(ib, (dst0, bucket_of_node[src0]), 1)
    # self-loops handled as diag (not edges) - not counted here
    nodes_by_core = [order_by_deg[core_of_node[order_by_deg] == c]
                     for c in range(N_CORES)]
    npcs = [len(n) for n in nodes_by_core]
    assert max(npcs) <= TPC * P
    vecs = np.zeros((N_CORES, max(npcs), 4), dtype=np.int64)
    nodemat = np.zeros((N_CORES, max(npcs)), dtype=np.int64)
    for c in range(N_CORES):
        nodemat[c, :npcs[c]] = nodes_by_core[c]
        vecs[c, :npcs[c]] = ib[nodes_by_core[c]]
    tile_local = _balance_tiles(vecs, nodemat)

    # ---- slots: node -> (core, tile_local, p)
    slot_of = np.empty(n_nodes, dtype=np.int64)
    tfill = np.zeros((N_CORES, TPC), dtype=np.int64)
    for c in range(N_CORES):
        tl = tile_local[c, :npcs[c]]
        nodes = nodemat[c, :npcs[c]]
        for i in range(npcs[c]):
            t = tl[i]
            slot_of[nodes[i]] = (c * TPC + t) * P + tfill[c, t]
            tfill[c, t] += 1
    assert tfill.max() <= P

    src = slot_of[src0].astype(np.int64)
    dst = slot_of[dst0].astype(np.int64)
    dinv_slot = np.zeros(n_slots, dtype=np.float32)
    dinv_slot[slot_of] = dinv

    gtile = dst >> 7
    core = gtile // TPC
    tl = gtile % TPC
    dstp = dst & (P - 1)
    # table row of a node slot (c, t, p): c*12544 + p*TPC + t
    sc = src // (TPC * P)
    st = (src // P) % TPC
    sp = src & (P - 1)
    table_row = sc * (TPC * P) + sp * TPC + st
    bucket = table_row // BROWS  # == sc // 2
    idx_rel_all = (table_row - bucket * BROWS).astype(np.int32)

    # ---- counts and uniform chunk schedule
    counts = np.zeros((N_CORES, TPC, N_BUCKETS), dtype=np.int64)
    np.add.at(counts, (core, tl, bucket), 1)
    nchunk_u = (np.ceil(counts.max(axis=0) / P)).astype(np.int64)  # [TPC, 4]
    nchunk_u = np.maximum(nchunk_u, 1)

    n_groups = (TPC + GT - 1) // GT
    chunk_tile = []
    gb_meta = []   # (g, b, chunk_start, nch, idx_col_start)
    slot_base = {}
    cpos = 0
    col_off = 0
    gb_nchunks = np.zeros((n_groups, N_BUCKETS), dtype=np.int64)
    for g in range(n_groups):
        t0, t1 = g * GT, min((g + 1) * GT, TPC)
        for b in range(N_BUCKETS):
            nch = 0
            for t in range(t0, t1):
                slot_base[(t, b)] = (cpos + nch) * P
                for _ in range(int(nchunk_u[t, b])):
                    chunk_tile.append(t)
                nch += int(nchunk_u[t, b])
            gb_meta.append((g, b, cpos, nch, col_off))
            gb_nchunks[g, b] = nch
            col_off += nch * P // 16
            cpos += nch
    nct = cpos
    chunk_tile = np.array(chunk_tile, dtype=np.int32)

    # ---- per-core edge placement
    order = np.lexsort((bucket, tl, core))
    s_c, s_t, s_b = core[order], tl[order], bucket[order]
    s_idx, s_dstp = idx_rel_all[order], dstp[order]

    idx_rel = np.zeros((N_CORES, nct * P), dtype=np.int16)  # pad: row 0, S=0
    dstloc = np.full((N_CORES, nct * P), 300.0, dtype=np.float16)
    key = (s_c * TPC + s_t) * N_BUCKETS + s_b
    startd = np.r_[True, key[1:] != key[:-1]]
    run_id = np.cumsum(startd) - 1
    run_start = np.nonzero(startd)[0]
    within = np.arange(len(key)) - run_start[run_id]
    base = np.array([slot_base[(int(t), int(b))]
                     for t, b in zip(s_t[startd], s_b[startd])])
    gpos = base[run_id] + within
    idx_rel[s_c, gpos] = s_idx.astype(np.int16)
    dstloc[s_c, gpos] = s_dstp.astype(np.float16)

    # wrapped idx layout: per (g,b) call, idx i -> [16 partitions, i//16], x8
    idx_cols_total = nct * P // 16
    idx_wrapped = np.zeros((N_CORES, P, idx_cols_total), dtype=np.int16)
    for (g, b, cpos0, nch, col0) in gb_meta:
        if nch == 0:
            continue
        ni = nch * P
        span = slice(cpos0 * P, cpos0 * P + ni)
        blk = idx_rel[:, span].reshape(N_CORES, ni // 16, 16)
        w = np.transpose(blk, (0, 2, 1))
        idx_wrapped[:, :, col0:col0 + ni // 16] = np.tile(w, (1, 8, 1))

    dstloc_T = np.transpose(dstloc.reshape(N_CORES, nct, P), (0, 2, 1)).copy()
    dinv_T = dinv_slot.reshape(N_CORES, TPC, P).transpose(0, 2, 1).copy()

    nchmax = int(gb_nchunks.max())
    io_mat = np.tile(np.arange(P, dtype=np.float16)[None, :, None],
                     (P, 1, nchmax)).copy()

    pad = nct * P / (len(src) / N_CORES) - 1
    return dict(
        dinv=dinv_slot, dinv_T=dinv_T, io_mat=io_mat, slot_of=slot_of,
        idx_wrapped=idx_wrapped, dstloc_T=dstloc_T,
        chunk_tile=chunk_tile, gb_meta=gb_meta, gb_nchunks=gb_nchunks,
        n_groups=n_groups, n_chunks_total=nct, nchmax=nchmax, pad=pad,
    )


def build_prep_kernel(feat, R=1, nsplit=7):
    """gT[p, t, f] = fp16(dinv[t,p] * xT[p, t, f]); pipelined big DMAs."""
    nc = bacc.Bacc("TRN2")
    nt = TPC
    q = nt // nsplit
    assert nt % nsplit == 0
    x = nc.dram_tensor("x", [P, nt * feat], mybir.dt.float16, kind="ExternalInput")
    dinvT = nc.dram_tensor("dinvT", [P, nt], mybir.dt.float32, kind="ExternalInput")
    g = nc.dram_tensor("g", [P, nt * feat], mybir.dt.float16, kind="ExternalOutput")
    with tile.TileContext(nc) as tc:
        with tc.tile_pool(name="sb", bufs=3) as pool, \
             tc.tile_pool(name="cst", bufs=1) as cpool:
            dv = cpool.tile([P, nt], mybir.dt.float32)
            nc.sync.dma_start(dv[:], dinvT[:])
            for _ in range(R):
                for s in range(nsplit):
                    t0 = s * q
                    xt = pool.tile([P, q, feat], mybir.dt.float16, tag="x")
                    nc.sync.dma_start(xt[:], x[:, t0*feat:(t0+q)*feat])
                    gt = pool.tile([P, q, feat], mybir.dt.float16, tag="g")
                    for t in range(q):
                        nc.vector.tensor_scalar(gt[:, t, :], xt[:, t, :],
                                                dv[:, t0+t:t0+t+1], None,
                                                mybir.AluOpType.mult)
                    nc.sync.dma_start(g[:, t0*feat:(t0+q)*feat], gt[:])
    nc.finalize()
    return nc


def build_conv_kernel(sched, feat_in, feat_out, out_fp16_scaled, R=1,
                      msg_bufs=8, s_bufs=4, ep_bufs=6, st_bufs=3):
    """One GCN conv layer (aggregate-first), v2.

    inputs: g [n_rows, feat_in] fp16 (table layout, row (c,t,p) at
            c*12544 + p*98 + t), gownT [128, TPC*feat_in] fp16,
            W [feat_in, feat_out] fp16, dinvT [128, TPC] fp32,
            io_mat [128, 128*nchmax] fp16, idxs [128, idx_cols] int16,
            dstlocT [128, nct] fp16
    output: outT [128, TPC*feat_out] (fp16 scaled by dinv, or fp32)
    """
    n_groups = sched["n_groups"]
    nct = sched["n_chunks_total"]
    chunk_tile = sched["chunk_tile"]
    gb_meta = sched["gb_meta"]
    nchmax = sched["nchmax"]
    idx_cols = sched["idx_wrapped"].shape[2]
    n_rows = TPC * P * N_CORES

    first_chunk = {}
    last_chunk = {}
    for ci, t in enumerate(chunk_tile):
        t = int(t)
        if t not in first_chunk:
            first_chunk[t] = ci
        last_chunk[t] = ci

    out_dtype = mybir.dt.float16 if out_fp16_scaled else mybir.dt.float32

    nc = bacc.Bacc("TRN2", num_swdge_queues=4)
    g = nc.dram_tensor("g", [n_rows, feat_in], mybir.dt.float16, kind="ExternalInput")
    gownT_t = nc.dram_tensor("gownT", [P, TPC * feat_in], mybir.dt.float16,
                             kind="ExternalInput")
    W = nc.dram_tensor("W", [feat_in, feat_out], mybir.dt.float16, kind="ExternalInput")
    dinvT = nc.dram_tensor("dinvT", [P, TPC], mybir.dt.float32, kind="ExternalInput")
    iomat_t = nc.dram_tensor("iomat", [P, P * nchmax], mybir.dt.float16,
                             kind="ExternalInput")
    ident_t = nc.dram_tensor("ident", [P, P], mybir.dt.float16,
                             kind="ExternalInput")
    idxs = nc.dram_tensor("idxs", [P, idx_cols], mybir.dt.int16, kind="ExternalInput")
    dstlocT = nc.dram_tensor("dstlocT", [P, nct], mybir.dt.float16,
                             kind="ExternalInput")
    out = nc.dram_tensor("out", [P, TPC * feat_out], out_dtype, kind="ExternalOutput")

    with tile.TileContext(nc) as tc:
        with tc.tile_pool(name="cst", bufs=1) as cpool, \
             tc.tile_pool(name="msg", bufs=msg_bufs) as mpool, \
             tc.tile_pool(name="sS", bufs=s_bufs) as spool, \
             tc.tile_pool(name="agg", bufs=1, space="PSUM") as apool, \
             tc.tile_pool(name="ops", bufs=2, space="PSUM") as opool, \
             tc.tile_pool(name="eps", bufs=ep_bufs) as epool, \
             tc.tile_pool(name="st", bufs=st_bufs) as stpool:
            w_sb = cpool.tile([feat_in, feat_out], mybir.dt.float16)
            nc.sync.dma_start(w_sb[:], W[:])
            dv = cpool.tile([P, TPC], mybir.dt.float32)
            nc.sync.dma_start(dv[:], dinvT[:])
            iom = cpool.tile([P, P, nchmax], mybir.dt.float16)
            nc.sync.dma_start(iom[:], iomat_t[:])
            idn = cpool.tile([P, P], mybir.dt.float16)
            nc.sync.dma_start(idn[:], ident_t[:])
            ix = cpool.tile([P, idx_cols], mybir.dt.int16)
            nc.sync.dma_start(ix[:], idxs[:])
            dl = cpool.tile([P, nct], mybir.dt.float16)
            nc.sync.dma_start(dl[:], dstlocT[:])
            gown = cpool.tile([P, TPC, feat_in], mybir.dt.float16)
            nc.sync.dma_start(gown[:], gownT_t[:])

            for _ in range(R):
                for gi in range(n_groups):
                    t0 = gi * GT
                    t1 = min((gi + 1) * GT, TPC)
                    banks = [apool.tile([P, 512], mybir.dt.float32, tag=f"agg{k}",
                                        name=f"aggb_{gi}_{k}")
                             for k in range(t1 - t0)]

                    def agg_slice(t):
                        return banks[t - t0][:, :P]

                    # self-loop diagonal: aggT[:, p] = gown[p, t, :]^T via PE
                    # transpose (lhsT read straight from preloaded SBUF)
                    for t in range(t0, t1):
                        nc.tensor.matmul(agg_slice(t), lhsT=gown[:, t, :],
                                         rhs=idn[:], start=True, stop=False)

                    for b in range(N_BUCKETS):
                        _, _, c_start, nch, col0 = gb_meta[gi * N_BUCKETS + b]
                        if nch == 0:
                            continue
                        msg = mpool.tile([P, nch, feat_in], mybir.dt.float16,
                                         tag="msg")
                        base = b * BROWS
                        rows = min(BROWS, n_rows - base)
                        nc.gpsimd.dma_gather(
                            msg[:], g[base:base + rows, :],
                            ix[:, col0:col0 + nch * P // 16],
                            nch * P, nch * P, feat_in,
                            single_packet=False,
                            queue_num=(gi * N_BUCKETS + b) % 4,
                        )
                        # S2[e, d, k] = (d == dstloc[e, c_start+k]); both
                        # operands innermost-packed -> DVE 2x
                        S2 = spool.tile([P, P, nch], mybir.dt.float16, tag="S")
                        dl_ap = dl[:, c_start:c_start + nch]
                        dl_r = dataclasses.replace(
                            dl_ap, ap=[dl_ap.ap[0], [0, P], dl_ap.ap[1]])
                        nc.vector.tensor_tensor(
                            S2[:], iom[:, :, :nch], dl_r,
                            mybir.AluOpType.is_equal)
                        for k in range(nch):
                            ci = c_start + k
                            t = int(chunk_tile[ci])
                            nc.tensor.matmul(
                                agg_slice(t), lhsT=msg[:, k, :],
                                rhs=S2[:, :, k],
                                start=False, stop=(ci == last_chunk[t]))

                    # epilogue: stage group outputs, one DMA per group
                    ot = stpool.tile([P, t1 - t0, feat_out], out_dtype, tag="ot")
                    for t in range(t0, t1):
                        aggsb = epool.tile([P, P], mybir.dt.float16, tag="aggsb")
                        nc.vector.tensor_copy(aggsb[:], agg_slice(t))
                        ops = opool.tile([P, feat_out], mybir.dt.float32, tag="ops")
                        nc.tensor.matmul(ops[:], lhsT=aggsb[:], rhs=w_sb[:],
                                         start=True, stop=True)
                        dvt = dv[:, t:t+1]
                        e = epool.tile([P, feat_out], mybir.dt.float32, tag="e")
                        nc.scalar.activation(e[:], ops[:],
                                             mybir.ActivationFunctionType.Exp,
                                             scale=dvt)
                        r = epool.tile([P, feat_out], mybir.dt.float32, tag="r")
                        nc.scalar.activation(r[:], e[:],
                                             mybir.ActivationFunctionType.Relu,
                                             bias=1.0, scale=-1.0)
                        p = epool.tile([P, feat_out], mybir.dt.float32, tag="p")
                        nc.scalar.activation(p[:], ops[:],
                                             mybir.ActivationFunctionType.Relu,
                                             scale=dvt)
                        if out_fp16_scaled:
                            elu = epool.tile([P, feat_out], mybir.dt.float32,
                                             tag="elu")
                            nc.vector.tensor_tensor(elu[:], p[:], r[:],
                                                    mybir.AluOpType.subtract)
                            nc.vector.tensor_scalar(ot[:, t - t0, :], elu[:],
                                                    dvt, None,
                                                    mybir.AluOpType.mult)
                        else:
                            nc.vector.tensor_tensor(ot[:, t - t0, :], p[:], r[:],
                                                    mybir.AluOpType.subtract)
                    nc.sync.dma_start(
                        out[:, t0 * feat_out:t1 * feat_out], ot[:])
    nc.finalize()
    return nc


import sys as _sys
import types as _types


def _ensure_axon_stub():
    try:
        import antenv.axon_hooks  # noqa
    except ModuleNotFoundError:
        try:
            import antenv
        except ModuleNotFoundError:
            antenv = _types.ModuleType("antenv")
            _sys.modules["antenv"] = antenv
        import antenv
        m = _types.ModuleType("antenv.axon_hooks")
        m.get_axon_ntff_profile_hook = lambda: None
        _sys.modules["antenv.axon_hooks"] = m
        antenv.axon_hooks = m


def _to_xT(x_rows):
    """[12544, feat] row-major (t,p) -> [128, TPC*feat] partition-major."""
    feat = x_rows.shape[1]
    return np.ascontiguousarray(
        x_rows.reshape(TPC, P, feat).transpose(1, 0, 2).reshape(P, TPC * feat))


def _from_outT(outT, feat):
    """[128, TPC*feat] -> [12544, feat] row-major (t,p)."""
    return np.ascontiguousarray(
        outT.reshape(P, TPC, feat).transpose(1, 0, 2).reshape(TPC * P, feat))


def kernel(x, edge_index, W1, b1, W2, b2):
    _ensure_axon_stub()
    from concourse.bass_utils import run_bass_kernel_spmd

    x = np.asarray(x, dtype=np.float32)
    edge_index = np.asarray(edge_index)
    W1 = np.asarray(W1, dtype=np.float32)
    W2 = np.asarray(W2, dtype=np.float32)
    assert np.all(np.asarray(b1) == 0) and np.all(np.asarray(b2) == 0)

    sched = build_schedule(edge_index, N_NODES)
    slot_of = sched["slot_of"]
    rows_pc = TPC * P
    cores = list(range(N_CORES))

    # ---- launch A: gT = fp16(dinv * x), node-sharded, transposed layout
    x_pad = np.zeros((rows_pc * N_CORES, IN_DIM), np.float32)
    x_pad[slot_of] = x
    nc_a = build_prep_kernel(IN_DIM)
    in_a = [{"x": _to_xT(x_pad[c*rows_pc:(c+1)*rows_pc]).astype(np.float16),
             "dinvT": sched["dinv_T"][c]} for c in cores]
    res_a = run_bass_kernel_spmd(nc_a, in_a, core_ids=cores, trace=False)
    gT = [res_a.results[c]["g"] for c in cores]  # [128, TPC*IN_DIM] each
    # global gather table: core c rows at c*12544, row (c,t,p) = c*12544+p*98+t
    g1 = np.concatenate([gt.reshape(rows_pc, IN_DIM) for gt in gT])

    ident = np.eye(P, dtype=np.float16)
    common = lambda c: {"dinvT": sched["dinv_T"][c],
                        "iomat": sched["io_mat"].reshape(P, -1),
                        "ident": ident,
                        "idxs": sched["idx_wrapped"][c],
                        "dstlocT": sched["dstloc_T"][c]}

    # ---- launch B: conv1 -> fp32 elu(.); host folds the dinv scale and
    # fp16 cast while assembling the next layer's gather table (the same
    # inter-launch relayout/broadcast step that ships g2 to every core)
    nc_b = build_conv_kernel(sched, IN_DIM, HID_DIM, out_fp16_scaled=False)
    in_b = [{"g": g1, "gownT": gT[c], "W": W1.astype(np.float16), **common(c)}
            for c in cores]
    res_b = run_bass_kernel_spmd(nc_b, in_b, core_ids=cores, trace=False)
    dinv_pt = sched["dinv_T"]  # [cores, P, TPC], [p, t] = dinv of slot (c,t,p)
    g2T = [(res_b.results[c]["out"].reshape(P, TPC, HID_DIM)
            * dinv_pt[c][:, :, None]).astype(np.float16)
           .reshape(P, TPC * HID_DIM) for c in cores]
    g2 = np.concatenate([gt.reshape(rows_pc, HID_DIM) for gt in g2T])

    # ---- launch C: conv2 -> fp32 elu(.)
    nc_c = build_conv_kernel(sched, HID_DIM, OUT_DIM, out_fp16_scaled=False)
    in_c = [{"g": g2, "gownT": g2T[c], "W": W2.astype(np.float16), **common(c)}
            for c in cores]
    res_c = run_bass_kernel_spmd(nc_c, in_c, core_ids=cores, trace=False)
    out = np.concatenate([_from_outT(res_c.results[c]["out"], OUT_DIM)
                          for c in cores])
    # rows are in (c,t,p) slot order = slot id; un-permute
    return np.ascontiguousarray(out[slot_of].astype(np.float32))
